# revision 20
# baseline (speedup 1.0000x reference)
"""Trainium2 Bass kernel v6 for Conv2Demod — Winograd F(2x2,3x3), bf16 PE.

Per-sample computation as Winograd:
    out = A^T [ (G w G^T) elemwise (B^T (d*(1+s)) B) ] A * d_o
  - Host folds the per-channel modulation (1+s_i) into the image and
    computes the demod vector d_o = rsqrt(sum (w*(1+s))^2 + eps)
    directly (cheap input prep); d_o is the PSUM eviction scale.
  - Host precomputes wg[uv][i,o] = (G W G^T), stored UV-MAJOR.

Device structure (one sample per core, 8 cores data-parallel), fully
QUARTER-granular software pipeline (quarter = 8 tile-rows = 256 tiles):

  for gq in 0..3:                       # quarter being consumed
    emit transform chunks of gq+1      # interleaved: DVE/Pool fill idle
    for ot in 0..3:                    # o-tile
      for ph in 0..1:                  # Winograd v-set phase
        32 matmuls into 8 PSUM banks x 256 cols (ping-pong parity, so
          consecutive phases use disjoint PSUM halves -> no WAR stalls)
        1 ACT eviction (N=2048, demod scale)
        pass1 on DVE (v-pairs, N=512)
      pass2 in-place in psb[ot]
      if gq odd: osb interleave (ACT) + DMA out for this o-tile/half

  - vt quarter tiles double-buffered: transform(gq+1) overlaps
    consume(gq); the For_i all-engine barrier separates trips, and a
    prologue transforms gq0 for trip 0 (the body's tail transforms
    next-trip gq0 during gq3 consumption).
  - Engine split: DVE: BT1(u<2), most BT2, pass1, pass2; Pool/GpSimd:
    BT1(u>=2) + a BT2 slice; ACT: evictions + osb; weights on the
    scalar DMA queue, image/output on sync.
"""

import contextlib

import numpy as np
import ml_dtypes

import concourse.bacc as bacc
import concourse.mybir as mybir
import concourse.tile as tile
from concourse.bass_utils import run_bass_kernel_spmd

P = 128
CIN = 512
COUT = 512
H = W = 64
NI = CIN // P
NO = COUT // P
T = H // 2          # 32 tile cols
TH = T // 2         # 16 tile rows per half
TQ = TH // 2        # 8 tile rows per quarter
NPIX = TH * T       # 512 tiles per half
NPQ = TQ * T        # 256 tiles per quarter
HP = H + 2
WP = W + 2          # 66 = 33 even + 33 odd x-positions
XE = WP // 2        # 33
EPS = 1e-8
N_CORES = 8

F32 = mybir.dt.float32
BF16 = mybir.dt.bfloat16
AF = mybir.ActivationFunctionType
ALU = mybir.AluOpType
_nullcm = contextlib.nullcontext

# phase -> v-set; bank k = 2*u + vset.index(v)
VSETS = ((0, 1), (2, 3))


def build_nc(loop_n=None):
    nc = bacc.Bacc("TRN2", target_bir_lowering=False, debug=False)

    img = nc.dram_tensor("img", [NI, P, HP, WP], BF16, kind="ExternalInput").ap()
    d_in = nc.dram_tensor("d", [COUT], F32, kind="ExternalInput").ap()
    wg = nc.dram_tensor("wg", [16, NI, P, COUT], BF16, kind="ExternalInput").ap()
    out = nc.dram_tensor("out", [COUT, H, W], F32, kind="ExternalOutput").ap()

    with tile.TileContext(nc) as tc:
        with (
            tc.tile_pool(name="const", bufs=1) as cpool,
            tc.tile_pool(name="wuv", bufs=1) as wpool,
            tc.tile_pool(name="imgh", bufs=5) as imghp,
            tc.tile_pool(name="v1", bufs=5) as v1p,
            tc.tile_pool(name="vt", bufs=2) as vtp,
            tc.tile_pool(name="msb", bufs=2) as msbp,
            tc.tile_pool(name="psb", bufs=1) as psbp,
            tc.tile_pool(name="osb", bufs=1) as osbp,
            tc.tile_pool(name="psum", bufs=1, space="PSUM") as psum_pool,
        ):
            # ---------------- prologue: runs once ----------------------
            dsb = cpool.tile([P, NO], F32, tag="dsb")
            nc.scalar.dma_start(dsb[:], d_in.rearrange("(t p) -> p t", p=P))
            wuv = [None] * 16
            uv_order = [4 * u + v for vs in VSETS for u in range(4)
                        for v in vs]
            for uv in uv_order:
                wm = wpool.tile([P, NI, COUT], BF16, tag=f"wuv{uv}")
                wuv[uv] = wm
                nc.scalar.dma_start(
                    wm[:], wg[uv].rearrange("t p o -> p t o")
                )

            # two PSUM tiles, ping-ponged per phase: separate tiles keep
            # the dependency tracking precise, so phase i+1's matmuls
            # never wait on phase i's eviction (WAR distance = 2 phases)
            ps2 = [
                psum_pool.tile([P, 8, NPQ], F32, tag=f"ps{i}",
                               name=f"ps{i}")
                for i in range(2)
            ]

            def tt(eng, o, a, b, op):
                eng.tensor_tensor(o, a, b, op)

            vt_cur = {}   # (slot 0..4, it) -> tile ; slot 4 = next-trip 0

            def transform_chunk(slot, gq, u):
                """One u-group of the input transform for quarter gq
                (written into vt slot `slot`).  u==0 also issues the
                imgh DMAs and allocates the vt tiles."""
                if u == 0:
                    imghs = []
                    for it in range(NI):
                        imgh = imghp.tile([P, 9, 2, WP], BF16, tag="imgh")
                        imghs.append(imgh)
                        nc.sync.dma_start(
                            imgh[:].rearrange("p a b x -> p (a b) x"),
                            img[it][:, 16 * gq : 16 * gq + 18, :],
                        )
                    transform_chunk.imghs = imghs
                    for it in range(NI):
                        vt_cur[(slot, it)] = vtp.tile(
                            [P, 16, TQ, T], BF16, tag=f"vt{it}",
                            name=f"vt{it}_s{slot}",
                        )
                imghs = transform_chunk.imghs
                for it in range(NI):
                    imgh = imghs[it]
                    d0 = imgh[:, 0:TQ, 0, :]
                    d1 = imgh[:, 0:TQ, 1, :]
                    d2 = imgh[:, 1 : TQ + 1, 0, :]
                    d3 = imgh[:, 1 : TQ + 1, 1, :]
                    # BT: u0=d0-d2, u1=d1+d2, u2=d2-d1, u3=d1-d3
                    upat = [
                        (d0, d2, ALU.subtract),
                        (d1, d2, ALU.add),
                        (d2, d1, ALU.subtract),
                        (d1, d3, ALU.subtract),
                    ][u]
                    v1 = v1p.tile([P, TQ, WP], BF16, tag="v1")
                    e1 = nc.gpsimd if u >= 2 else nc.vector
                    tt(e1, v1[:], upat[0], upat[1], upat[2])
                    x0 = v1[:, :, 0:T]            # even x: 0..62
                    x2 = v1[:, :, 1 : T + 1]      # even x: 2..64
                    x1 = v1[:, :, XE : XE + T]    # odd x: 1..63
                    x3 = v1[:, :, XE + 1 : XE + T + 1]  # odd x: 3..65
                    vpat = [
                        (x0, x2, ALU.subtract),
                        (x1, x2, ALU.add),
                        (x2, x1, ALU.subtract),
                        (x1, x3, ALU.subtract),
                    ]
                    e2 = nc.gpsimd if (
                        (u == 3 and it <= 1) or (u == 2 and it == 3)
                    ) else nc.vector
                    for v, (xa, xb, xop) in enumerate(vpat):
                        tt(e2, vt_cur[(slot, it)][:, u * 4 + v],
                           xa, xb, xop)

            # prologue: transform quarter 0 into slot 0
            for u in range(4):
                transform_chunk(0, 0, u)

            psbs = [None] * NO

            with (tc.For_i(0, loop_n, 1) if loop_n else _nullcm()):
                phase_ctr = 0
                for gq in range(4):
                    qh = gq % 2            # position within half
                    h = gq // 2
                    nslot = gq + 1         # producing next quarter
                    for ot in range(NO):
                        # interleave next quarter's transform emission
                        transform_chunk(nslot, (gq + 1) % 4, ot)
                        o0 = ot * P
                        msb = msbp.tile(
                            [P, 2, 8, NPQ], BF16, tag="msb",
                            name=f"msb_{gq}_{ot}",
                        )
                        if gq == 0 and ot == 0:
                            pass  # psb allocated below per ot
                        if qh == 0:
                            psbs[ot] = psbp.tile(
                                [P, 2, 4, NPIX], BF16, tag=f"psb{ot}",
                                name=f"psb{ot}_{h}",
                            )
                        psb = psbs[ot]
                        qs = slice(NPQ * qh, NPQ * (qh + 1))
                        for ph in range(2):
                            ps = ps2[phase_ctr % 2]
                            phase_ctr += 1
                            vset = VSETS[ph]
                            uvs = [4 * u + v for u in range(4) for v in vset]
                            for k, uv in enumerate(uvs):
                                for it in range(NI):
                                    nc.tensor.matmul(
                                        ps[:, k, :],
                                        wuv[uv][:, it, o0 : o0 + P],
                                        vt_cur[(gq, it)][:, uv].rearrange(
                                            "p t c -> p (t c)"),
                                        start=(it == 0),
                                        stop=(it == NI - 1),
                                    )
                            # whole-phase eviction (ping-pong makes the
                            # next phase independent of it)
                            nc.scalar.activation(
                                msb[:, ph, :, :].rearrange(
                                    "p k n -> p (k n)"),
                                ps[:].rearrange("p k n -> p (k n)"),
                                AF.Copy, scale=dsb[:, ot : ot + 1],
                            )
                        # pass1 (y-dir) merged across both phases:
                        # v = 2*ph + vi, so msb[:, :, 2u:2u+2, :] iterates
                        # (ph, vi, col) = (v, col) -> psb[:, a, 0:4, qs]
                        # AT = [[1,1,1,0],[0,1,-1,-1]] over u
                        mU = [msb[:, :, 2 * u : 2 * u + 2, :]
                              for u in range(4)]
                        pA = psb[:, 0, 0:4, qs].rearrange(
                            "p (w v) n -> p w v n", w=2)
                        pB = psb[:, 1, 0:4, qs].rearrange(
                            "p (w v) n -> p w v n", w=2)
                        tt(nc.vector, pA, mU[0], mU[1], ALU.add)
                        tt(nc.vector, pA, pA, mU[2], ALU.add)
                        tt(nc.vector, pB, mU[1], mU[2], ALU.subtract)
                        tt(nc.vector, pB, pB, mU[3], ALU.subtract)
                        # pass2 (x-dir) in-place in psb, merged across a
                        # (slots v0<-y0, v1<-y1; P0 dead after op1, P1
                        # read before its overwrite)
                        Pv = [psb[:, :, v, qs] for v in range(4)]
                        tt(nc.vector, Pv[0], Pv[0], Pv[1], ALU.add)
                        tt(nc.vector, Pv[1], Pv[1], Pv[2], ALU.subtract)
                        tt(nc.vector, Pv[0], Pv[0], Pv[2], ALU.add)
                        tt(nc.vector, Pv[1], Pv[1], Pv[3], ALU.subtract)
                        if qh == 1:
                            # interleave + convert to f32, then DMA out
                            osb = osbp.tile(
                                [P, TH, 2, T, 2], F32, tag="osb",
                                name=f"osb_{h}_{ot}",
                            )
                            for a in range(2):
                                nc.scalar.activation(
                                    osb[:, :, a, :, :],
                                    psb[:, a, 0:2, :].rearrange(
                                        "p b n -> p n b"),
                                    AF.Copy,
                                )
                            nc.sync.dma_start(
                                out[o0 : o0 + P,
                                    2 * TH * h : 2 * TH * (h + 1), :],
                                osb[:].rearrange("p t a c b -> p (t a) (c b)"),
                            )
                # shift: next-trip slot 4 becomes slot 0
                for it in range(NI):
                    vt_cur[(0, it)] = vt_cur[(4, it)]
    nc.compile()
    return nc


_NC_CACHE = None


def _get_nc():
    global _NC_CACHE
    if _NC_CACHE is None:
        _NC_CACHE = build_nc()
    return _NC_CACHE


_G = np.array(
    [[1, 0, 0], [0.5, 0.5, 0.5], [0.5, -0.5, 0.5], [0, 0, 1]], np.float64
)


def make_in_maps(img, s, weight):
    img = np.asarray(img, dtype=np.float32)
    s = np.ascontiguousarray(np.asarray(s, dtype=np.float32))
    weight = np.asarray(weight, dtype=np.float32)
    b = img.shape[0]
    smod = 1.0 + s                                    # [B, CIN]
    # fold modulation into the image; zero-pad; DE-INTERLEAVE x
    imgm = img * smod[:, :, None, None]
    imgp = np.zeros((b, NI, P, HP, WP), dtype=np.float32)
    imgp[:, :, :, 1 : H + 1, 1 : W + 1] = imgm.reshape(b, NI, P, H, W)
    imgd = np.concatenate(
        [imgp[..., 0::2], imgp[..., 1::2]], axis=-1
    ).astype(ml_dtypes.bfloat16)
    # wg[uv][i, o] = (G W G^T)[o,i,u,v], uv-major -> [16, NI, P, COUT]
    wgf = np.einsum("ua,oiab,vb->uvio", _G, weight.astype(np.float64), _G)
    wgt = np.ascontiguousarray(
        wgf.reshape(16, NI, P, COUT).astype(ml_dtypes.bfloat16)
    )
    # demod d[b, o] = rsqrt(sum_i w2[i, o] * (1+s)^2 + eps)
    w2 = (weight.astype(np.float64) ** 2).sum(axis=(2, 3))  # [O, I]
    dvec = 1.0 / np.sqrt(
        (smod.astype(np.float64) ** 2) @ w2.T + EPS
    )                                                  # [B, O]
    dvec = dvec.astype(np.float32)
    return [
        {"img": imgd[i], "d": dvec[i], "wg": wgt} for i in range(b)
    ]


def kernel(img, s, weight):
    nc = _get_nc()
    in_maps = make_in_maps(img, s, weight)
    res = run_bass_kernel_spmd(nc, in_maps, list(range(N_CORES)))
    return np.stack([res.results[b]["out"] for b in range(N_CORES)], axis=0)


# revision 37
# speedup vs baseline: 10.3374x; 10.3374x over previous
"""Trainium2 Bass kernel v8 for Conv2Demod — Winograd F(2x2,3x3), bf16 PE.

Per-sample computation as Winograd:
    out = A^T [ (G w G^T) elemwise (B^T (d*(1+s)) B) ] A * d_o
  - Host folds the per-channel modulation (1+s_i) into the image and
    computes the demod vector d_o = rsqrt(sum (w*(1+s))^2 + eps)
    directly (cheap input prep); d_o is the PSUM eviction scale.
  - Host precomputes wg[uv][i,o] = (G W G^T), stored UV-MAJOR so weight
    planes stream in exactly the order the PE consumes them.
  - Output is written bf16 and upcast to f32 on the host (adds ~0.2%
    rms against a 2e-2 budget; halves output DMA traffic).

Device structure (one sample per core, 8 cores data-parallel), fully
QUARTER-granular pipeline (quarter = 8 tile-rows = 256 tiles), with a
SELF-CONTAINED For_i body (cross-boundary tile flows serialize under
the loop's semaphore reset):

  transform(q0)                         # ramp
  for gq in 0..3:
    for ot in 0..3:
      [interleaved: transform chunk u=ot of quarter gq+1]
      for ph in 0..1:
        32 matmuls into PSUM tile ps[parity] (8 banks x 256 cols);
          the two PSUM tiles ping-pong per phase, so phase i+1 never
          waits on phase i's eviction (separate tiles keep Tile's
          dependency tracking precise)
        1 ACT eviction (N=2048, demod scale folded in)
      pass1 on DVE merged across phases (u-pairs, N=1024) -> psb
      pass2 in-place in psb (a-merged, N=512, packed bf16)
      osb: one ACT 4D-AP op interleaves (a,b) -> (y,x) in bf16
      DMA out rows [16*gq, 16*gq+16) on the scalar queue

  - vt quarter tiles double-buffered: transform(gq+1) overlaps
    consume(gq) on the idle slices of DVE/Pool; BT2 emitted v-major so
    PSUM banks become consumable in PE order.
  - Engine split: DVE: BT1(u<2), most BT2, pass1, pass2; Pool/GpSimd:
    BT1(u>=2) + a BT2 slice; ACT: evictions + osb interleave; weights
    + output DMAs ride the scalar queue, image DMAs the sync queue.
"""

import contextlib

import numpy as np
import ml_dtypes

import concourse.bacc as bacc
import concourse.mybir as mybir
import concourse.tile as tile
from concourse.bass_utils import run_bass_kernel_spmd

P = 128
CIN = 512
COUT = 512
H = W = 64
NI = CIN // P
NO = COUT // P
T = H // 2          # 32 tile cols
TH = T // 2         # 16 tile rows per half
TQ = TH // 2        # 8 tile rows per quarter
NPIX = TH * T       # 512 tiles per half
NPQ = TQ * T        # 256 tiles per quarter
HP = H + 2
WP = W + 2          # 66 = 33 even + 33 odd x-positions
XE = WP // 2        # 33
EPS = 1e-8
N_CORES = 8

F32 = mybir.dt.float32
BF16 = mybir.dt.bfloat16
AF = mybir.ActivationFunctionType
ALU = mybir.AluOpType
_nullcm = contextlib.nullcontext

# phase -> v-set; bank k = 2*u + vset.index(v)
VSETS = ((0, 1), (2, 3))


def build_nc(loop_n=None):
    nc = bacc.Bacc("TRN2", target_bir_lowering=False, debug=False)

    img = nc.dram_tensor("img", [NI, P, HP, WP], BF16, kind="ExternalInput").ap()
    d_in = nc.dram_tensor("d", [COUT], F32, kind="ExternalInput").ap()
    wg = nc.dram_tensor("wg", [16, NI, P, COUT], BF16, kind="ExternalInput").ap()
    out = nc.dram_tensor("out", [COUT, H, W], BF16, kind="ExternalOutput").ap()

    with tile.TileContext(nc) as tc:
        with (
            tc.tile_pool(name="const", bufs=1) as cpool,
            tc.tile_pool(name="wuv", bufs=1) as wpool,
            tc.tile_pool(name="imgh", bufs=6) as imghp,
            tc.tile_pool(name="v1", bufs=16) as v1p,
            tc.tile_pool(name="vt", bufs=2) as vtp,
            tc.tile_pool(name="msb", bufs=3) as msbp,
            tc.tile_pool(name="psb", bufs=3) as psbp,
            tc.tile_pool(name="osb", bufs=2) as osbp,
            tc.tile_pool(name="psum", bufs=1, space="PSUM") as psum_pool,
        ):
            with (tc.For_i(0, loop_n, 1) if loop_n else _nullcm()):
                dsb = cpool.tile([P, NO], F32, tag="dsb")
                nc.scalar.dma_start(
                    dsb[:], d_in.rearrange("(t p) -> p t", p=P)
                )
                # two PSUM tiles, ping-ponged per phase
                ps2 = [
                    psum_pool.tile([P, 8, NPQ], F32, tag=f"ps{i}",
                                   name=f"ps{i}")
                    for i in range(2)
                ]

                def tt(eng, o, a, b, op):
                    eng.tensor_tensor(o, a, b, op)

                vt_cur = {}   # (gq, it) -> tile

                def bt1(gq, u, v1s_all):
                    """BT1 for one u-group; u==0 also issues imgh DMAs
                    (split across the sync and scalar queues) and
                    allocates the vt tiles."""
                    if u == 0:
                        imghs = []
                        for it in range(NI):
                            imgh = imghp.tile([P, 9, 2, WP], BF16, tag="imgh")
                            imghs.append(imgh)
                            ieng = nc.sync if it < 2 else nc.scalar
                            ieng.dma_start(
                                imgh[:].rearrange("p a b x -> p (a b) x"),
                                img[it][:, 16 * gq : 16 * gq + 18, :],
                            )
                        bt1.imghs = imghs
                        for it in range(NI):
                            vt_cur[(gq, it)] = vtp.tile(
                                [P, 16, TQ, T], BF16, tag=f"vt{it}",
                                name=f"vt{it}_{gq}",
                            )
                    imghs = bt1.imghs
                    v1s = []
                    for it in range(NI):
                        imgh = imghs[it]
                        d0 = imgh[:, 0:TQ, 0, :]
                        d1 = imgh[:, 0:TQ, 1, :]
                        d2 = imgh[:, 1 : TQ + 1, 0, :]
                        d3 = imgh[:, 1 : TQ + 1, 1, :]
                        # BT: u0=d0-d2, u1=d1+d2, u2=d2-d1, u3=d1-d3
                        upat = [
                            (d0, d2, ALU.subtract),
                            (d1, d2, ALU.add),
                            (d2, d1, ALU.subtract),
                            (d1, d3, ALU.subtract),
                        ][u]
                        v1 = v1p.tile([P, TQ, WP], BF16, tag="v1")
                        v1s.append(v1)
                        e1 = nc.gpsimd if u >= 2 else nc.vector
                        tt(e1, v1[:], upat[0], upat[1], upat[2])
                    v1s_all[u] = v1s

                def bt2(gq, u, vpair, v1s_all):
                    """BT2 for one (u, v-pair); v01 planes (phase 0) are
                    emitted for every u before any v23 plane so PSUM
                    banks become consumable in PE order."""
                    for v in vpair:
                        for it in range(NI):
                            v1 = v1s_all[u][it]
                            x0 = v1[:, :, 0:T]            # even x: 0..62
                            x2 = v1[:, :, 1 : T + 1]      # even x: 2..64
                            x1 = v1[:, :, XE : XE + T]    # odd x: 1..63
                            x3 = v1[:, :, XE + 1 : XE + T + 1]  # odd 3..65
                            xa, xb, xop = [
                                (x0, x2, ALU.subtract),
                                (x1, x2, ALU.add),
                                (x2, x1, ALU.subtract),
                                (x1, x3, ALU.subtract),
                            ][v]
                            e2 = nc.gpsimd if (
                                (u == 3 and it <= 1) or (u == 2 and it == 3)
                            ) else nc.vector
                            tt(e2, vt_cur[(gq, it)][:, u * 4 + v],
                               xa, xb, xop)

                # transform emission sub-chunks, in PE consumption order:
                # all (u, v01) first, then all (u, v23)
                def transform_subchunks(gq, v1s_all):
                    yield lambda: (bt1(gq, 0, v1s_all),
                                   bt2(gq, 0, (0, 1), v1s_all),
                                   bt1(gq, 1, v1s_all),
                                   bt2(gq, 1, (0, 1), v1s_all))
                    yield lambda: (bt1(gq, 2, v1s_all),
                                   bt2(gq, 2, (0, 1), v1s_all),
                                   bt1(gq, 3, v1s_all),
                                   bt2(gq, 3, (0, 1), v1s_all))
                    yield lambda: (bt2(gq, 0, (2, 3), v1s_all),
                                   bt2(gq, 1, (2, 3), v1s_all))
                    yield lambda: (bt2(gq, 2, (2, 3), v1s_all),
                                   bt2(gq, 3, (2, 3), v1s_all))

                # weight DMAs split across queues by consumption
                # deadline: scalar takes the earliest-needed planes,
                # gpsimd (SWDGE, emitted before any Pool BT1 work can
                # block its descriptor generation) the latest, and sync
                # a middle share emitted after gq0's imgh tiles
                wuv = [None] * 16
                uv_order = [4 * u + v for vs in VSETS for u in range(4)
                            for v in vs]
                for uv in uv_order:
                    wuv[uv] = wpool.tile(
                        [P, NI, COUT], BF16, tag=f"wuv{uv}",
                        name=f"wuv{uv}",
                    )

                def wuv_dma(eng, ks):
                    for k in ks:
                        uv = uv_order[k]
                        eng.dma_start(
                            wuv[uv][:], wg[uv].rearrange("t p o -> p t o")
                        )


                # ramp: transform quarter 0 up-front
                v1s_r = {}
                for fn in transform_subchunks(0, v1s_r):
                    fn()

                # weight stream after the ramp-transform emission so the
                # scalar queue serves gq0's imgh tiles first
                wuv_dma(nc.scalar, range(0, 16))


                phase_ctr = 0
                for gq in range(4):
                    chunks = (
                        list(transform_subchunks(gq + 1, {})) if gq < 3
                        else [None] * 4
                    )
                    for ot in range(NO):
                        # interleave next quarter's transform emission
                        if chunks[ot] is not None:
                            chunks[ot]()
                        o0 = ot * P
                        msb = msbp.tile(
                            [P, 2, 8, NPQ], BF16, tag="msb",
                            name=f"msb_{gq}_{ot}",
                        )
                        psb = psbp.tile(
                            [P, 2, 4, NPQ], BF16, tag="psb",
                            name=f"psb_{gq}_{ot}",
                        )
                        for ph in range(2):
                            ps = ps2[phase_ctr % 2]
                            phase_ctr += 1
                            vset = VSETS[ph]
                            uvs = [4 * u + v for u in range(4) for v in vset]
                            for k, uv in enumerate(uvs):
                                for it in range(NI):
                                    nc.tensor.matmul(
                                        ps[:, k, :],
                                        wuv[uv][:, it, o0 : o0 + P],
                                        vt_cur[(gq, it)][:, uv].rearrange(
                                            "p t c -> p (t c)"),
                                        start=(it == 0),
                                        stop=(it == NI - 1),
                                    )
                            # whole-phase eviction (ping-pong makes the
                            # next phase independent of it)
                            nc.scalar.activation(
                                msb[:, ph, :, :].rearrange(
                                    "p k n -> p (k n)"),
                                ps[:].rearrange("p k n -> p (k n)"),
                                AF.Copy, scale=dsb[:, ot : ot + 1],
                            )
                            if gq == 3 and ot == NO - 1:
                                # tail: per-phase pass1 so the final
                                # output chain starts one phase earlier
                                mUp = [msb[:, ph, 2 * u : 2 * u + 2, :]
                                       for u in range(4)]
                                pAp = psb[:, 0, 2 * ph : 2 * ph + 2, :]
                                pBp = psb[:, 1, 2 * ph : 2 * ph + 2, :]
                                tt(nc.vector, pAp, mUp[0], mUp[1], ALU.add)
                                tt(nc.vector, pAp, pAp, mUp[2], ALU.add)
                                tt(nc.vector, pBp, mUp[1], mUp[2],
                                   ALU.subtract)
                                tt(nc.vector, pBp, pBp, mUp[3],
                                   ALU.subtract)
                        if not (gq == 3 and ot == NO - 1):
                            # pass1 (y-dir) merged across both phases:
                            # v = 2*ph + vi -> psb[:, a, 0:4, :]
                            # AT = [[1,1,1,0],[0,1,-1,-1]] over u
                            mU = [msb[:, :, 2 * u : 2 * u + 2, :]
                                  for u in range(4)]
                            pA = psb[:, 0, 0:4, :].rearrange(
                                "p (w v) n -> p w v n", w=2)
                            pB = psb[:, 1, 0:4, :].rearrange(
                                "p (w v) n -> p w v n", w=2)
                            tt(nc.vector, pA, mU[0], mU[1], ALU.add)
                            tt(nc.vector, pA, pA, mU[2], ALU.add)
                            tt(nc.vector, pB, mU[1], mU[2], ALU.subtract)
                            tt(nc.vector, pB, pB, mU[3], ALU.subtract)
                        # pass2 (x-dir) in-place in psb, merged across a
                        # (slots v0<-y0, v1<-y1; P0 dead after op1, P1
                        # read before its overwrite)
                        Pv = [psb[:, :, v, :] for v in range(4)]
                        tt(nc.vector, Pv[0], Pv[0], Pv[1], ALU.add)
                        tt(nc.vector, Pv[1], Pv[1], Pv[2], ALU.subtract)
                        tt(nc.vector, Pv[0], Pv[0], Pv[2], ALU.add)
                        tt(nc.vector, Pv[1], Pv[1], Pv[3], ALU.subtract)
                        # osb: interleave (a, b, t, c) -> (t, a, c, b),
                        # one 4D-AP ACT op, bf16
                        osb = osbp.tile(
                            [P, TQ, 2, T, 2], BF16, tag="osb",
                            name=f"osb_{gq}_{ot}",
                        )
                        for a in range(2):
                            nc.scalar.activation(
                                osb[:, :, a, :, :].rearrange(
                                    "p t c b -> p b t c"),
                                psb[:, a, 0:2, :].rearrange(
                                    "p b (t c) -> p b t c", c=T),
                                AF.Copy,
                            )
                        nc.scalar.dma_start(
                            out[o0 : o0 + P,
                                16 * gq : 16 * (gq + 1), :],
                            osb[:].rearrange("p t a c b -> p (t a) (c b)"),
                        )
    nc.compile()
    return nc


_NC_CACHE = None


def _get_nc():
    global _NC_CACHE
    if _NC_CACHE is None:
        _NC_CACHE = build_nc()
    return _NC_CACHE


_G = np.array(
    [[1, 0, 0], [0.5, 0.5, 0.5], [0.5, -0.5, 0.5], [0, 0, 1]], np.float64
)


def make_in_maps(img, s, weight):
    img = np.asarray(img, dtype=np.float32)
    s = np.ascontiguousarray(np.asarray(s, dtype=np.float32))
    weight = np.asarray(weight, dtype=np.float32)
    b = img.shape[0]
    smod = 1.0 + s                                    # [B, CIN]
    # fold modulation into the image; zero-pad; DE-INTERLEAVE x
    imgm = img * smod[:, :, None, None]
    imgp = np.zeros((b, NI, P, HP, WP), dtype=np.float32)
    imgp[:, :, :, 1 : H + 1, 1 : W + 1] = imgm.reshape(b, NI, P, H, W)
    imgd = np.concatenate(
        [imgp[..., 0::2], imgp[..., 1::2]], axis=-1
    ).astype(ml_dtypes.bfloat16)
    # wg[uv][i, o] = (G W G^T)[o,i,u,v], uv-major -> [16, NI, P, COUT]
    wgf = np.einsum("ua,oiab,vb->uvio", _G, weight.astype(np.float64), _G)
    wgt = np.ascontiguousarray(
        wgf.reshape(16, NI, P, COUT).astype(ml_dtypes.bfloat16)
    )
    # demod d[b, o] = rsqrt(sum_i w2[i, o] * (1+s)^2 + eps)
    w2 = (weight.astype(np.float64) ** 2).sum(axis=(2, 3))  # [O, I]
    dvec = 1.0 / np.sqrt(
        (smod.astype(np.float64) ** 2) @ w2.T + EPS
    )                                                  # [B, O]
    dvec = dvec.astype(np.float32)
    return [
        {"img": imgd[i], "d": dvec[i], "wg": wgt} for i in range(b)
    ]


def kernel(img, s, weight):
    nc = _get_nc()
    in_maps = make_in_maps(img, s, weight)
    res = run_bass_kernel_spmd(nc, in_maps, list(range(N_CORES)))
    return np.stack(
        [np.asarray(res.results[b]["out"]).astype(np.float32)
         for b in range(N_CORES)],
        axis=0,
    )
